# revision 3
# baseline (speedup 1.0000x reference)
"""Trainium2 Bass kernel for LocalizationLoss (box MSE + cross-entropy, batch mean).

Input : output [262144, 1004] f32  (cols 0:4 = box pred cx,cy,w,h; cols 4:1004 = logits)
        target [262144, 5]    f32  (xmin,ymin,xmax,ymax,class_id)
Output: scalar f32 = mean_b( mean_4((box_pred-box_true)^2) + CE(logits, class) )

Strategy (pure data parallel over 8 cores, 32768 rows each):
  - rows mapped p-major: partition p owns rows p*256..p*256+255 of its shard
  - stream 38 variable-size groups of row-tiles [128, gs, 1004]; one DMA per
    group, alternating between the SP and ACT HWDGE rings (two parallel DMA
    FIFO streams keep all 16 SDMA engines uniformly fed and hide per-DMA
    completion latency); triggers issued one group ahead of compute;
    data pool is 5 buffers deep so fast DMA engines never starve behind a
    straggling engine's backlog
  - ScalarE: exp over logits with fused row-sum accumulator (sumexp)
  - picks (logits[row, class]): one DVE scalar_tensor_tensor per tile:
    (iota is_equal class) * logits with accum_out  (GpSimd cannot take
    these: TensorScalarPtr and TT-is_equal don't lower on Pool, and
    InstIndirectCopy crashes the Q7 on this image)
  - box loss: per group one GpSimd copy of the 4 box-pred cols into a
    persistent buffer; per supergroup of 6-9 groups, 6 TensorTensor ops +
    square + DVE reduce produce the (2*err)^2 sum (batching kills the
    ~0.9us per-op GpSimd overhead); the last tiny supergroup runs on DVE
    so GpSimd is never on the post-stream drain path
  - epilogue split: Ln(sumexp) + pick-sum over cols [0,192) run mid-stream
    (hidden in engine idle); only cols [192,256) drain at the end; the
    [128,1] partials are 32-block-transposed so the output DMA is 4
    contiguous 128B runs instead of 128 scattered 4B HBM read-modify-writes
  - each core returns [4,32] per-partition partial sums; host adds and /B

This container's walrus build accepts at most ONE sync-wait per instruction,
while the Tile scheduler attaches several. `_split_multiwait_bir` rewrites the
serialized BIR to hoist extra waits onto single-wait NoOp carriers, and is
installed as a wrapper around compile_bir_kernel at import time. The same
walrus also cannot lower the custom-DVE ISA ops (tensor_mask_reduce etc.) or
Pool-engine TensorScalarPtr, so only standard opcodes are used.
"""

import json as _json

import numpy as np

import concourse.bass as bass
import concourse.tile as tile
from concourse import mybir
import concourse.bass_utils as _bass_utils
import concourse.bass2jax as _bass2jax
from concourse.bass_utils import run_bass_kernel_spmd

P = 128
B = 262144
C = 1004
NCLS = 1000
NCORES = 8
R = B // NCORES       # 32768 rows per core
T = R // P            # 256 row-tiles per core (rows per partition)

F32 = mybir.dt.float32
ALU = mybir.AluOpType
ACTF = mybir.ActivationFunctionType

# NOTE: offloading picks to GpSimd is impossible in this walrus build:
# scalar_tensor_tensor lowers to TensorScalarPtr (rejected on Pool), plain
# TensorTensor is_equal is also rejected on Pool, and InstIndirectCopy
# (gather) compiles but crashes the Q7 exec unit on hardware. All picks
# therefore run on DVE as one fused compare-mult-accumulate each.


# --------------------------------------------------------------------------
# BIR post-pass: this image's walrus supports only one sync-wait per
# instruction; split extras onto NoOp carriers placed just before.
# --------------------------------------------------------------------------
def _split_multiwait_bir(bir_json: bytes) -> bytes:
    d = _json.loads(bir_json)
    changed = False
    for fn in d.get("functions", []):
        for blk in fn.get("blocks", []):
            insts = blk.get("instructions", [])
            out = []
            for ins in insts:
                si = ins.get("sync_info") or {}
                waits = si.get("on_wait") or []
                if len(waits) > 1:
                    changed = True
                    for i, w in enumerate(waits[:-1]):
                        out.append(
                            {
                                "debug": ins.get("debug", 0),
                                "engine": ins["engine"],
                                "ins": [],
                                "name": f"{ins['name']}-wsplit{i}",
                                "opcode": "NoOp",
                                "outs": [],
                                "sync_info": {"on_update": [], "on_wait": [w]},
                            }
                        )
                    ins["sync_info"]["on_wait"] = [waits[-1]]
                out.append(ins)
            blk["instructions"] = out
    if not changed:
        return bir_json
    return _json.dumps(d).encode()


_orig_compile_bir_kernel = _bass_utils.compile_bir_kernel


def _compile_bir_kernel_fixed(bir_json, tmpdir, neff_name="file.neff"):
    if isinstance(bir_json, str):
        bir_json = bir_json.encode()
    return _orig_compile_bir_kernel(_split_multiwait_bir(bir_json), tmpdir, neff_name)


if _bass_utils.compile_bir_kernel is not _compile_bir_kernel_fixed:
    _bass_utils.compile_bir_kernel = _compile_bir_kernel_fixed
    _bass2jax.compile_bir_kernel = _compile_bir_kernel_fixed


# --------------------------------------------------------------------------
# kernel build
# --------------------------------------------------------------------------
def build():
    nc = bass.Bass()
    x = nc.dram_tensor("x", [R, C], F32, kind="ExternalInput")
    t = nc.dram_tensor("t", [R, 5], F32, kind="ExternalInput")
    out = nc.dram_tensor("partial", [4, 32], F32, kind="ExternalOutput")

    xv = x[:].rearrange("(p n) c -> p n c", p=P)   # [128, 256, 1004]
    tv = t[:].rearrange("(p n) f -> p n f", p=P)   # [128, 256, 5]

    with tile.TileContext(nc) as tc:
        with (
            tc.tile_pool(name="data", bufs=5) as data_pool,
            tc.tile_pool(name="scr", bufs=2) as scr_pool,
            tc.tile_pool(name="acc", bufs=1) as acc_pool,
        ):
            # iota constant generated on-chip (saves a 512KB HBM read)
            iota_t = acc_pool.tile([P, NCLS], F32)
            nc.gpsimd.iota(
                iota_t,
                pattern=[[1, NCLS]],
                base=0,
                channel_multiplier=0,
                allow_small_or_imprecise_dtypes=True,
            )
            # whole per-core target resident: [128, 256, 5] = 5 KiB/partition;
            # issued on the ACT HWDGE ring so it overlaps group-0 data on SP
            tgt = acc_pool.tile([P, T, 5], F32)
            nc.scalar.dma_start(out=tgt, in_=tv)

            # variable group sizes: small head groups shrink the pipeline
            # fill, small tail groups shrink the end-of-run compute drain
            group_sizes = [1, 1, 2, 4] + [8] * 30 + [4, 2, 1, 1]
            assert sum(group_sizes) == T
            n_groups = len(group_sizes)
            starts = [0]
            for gs in group_sizes:
                starts.append(starts[-1] + gs)
            # supergroups for batched box math (last one handled on DVE and
            # kept tiny so GpSimd never sits on the post-stream drain path)
            supergroups = [(0, 7), (7, 13), (13, 19), (19, 25), (25, 34),
                           (34, 38)]
            n_sg = len(supergroups)
            # epilogue split: Ln / pick-sum over cols [0, EPI_SPLIT) run
            # mid-stream (engines are stream-paced and have idle); only the
            # tail [EPI_SPLIT, T) remains on the drain path
            EPI_SPLIT = 192
            epi_grp = next(
                i for i in range(n_groups) if starts[i + 1] == EPI_SPLIT
            )

            sumexp_all = acc_pool.tile([P, T], F32)   # per-row sum(exp(logits))
            picked_all = acc_pool.tile([P, T], F32)   # per-row logits[class]
            box_all = acc_pool.tile([P, T, 4], F32)   # box pred cols staging
            loc_sg = acc_pool.tile([P, n_sg], F32)    # per-supergroup sq-err
            logz_scr = acc_pool.tile([P, T], F32)
            logz_sumA = acc_pool.tile([P, 1], F32)
            logz_sumB = acc_pool.tile([P, 1], F32)
            pick_sumA = acc_pool.tile([P, 1], F32)
            pick_sumB = acc_pool.tile([P, 1], F32)

            # data DMAs alternate between the two HWDGE rings (SP / ACT)
            data_tiles = [None] * n_groups

            def issue_dma(grp):
                gs = group_sizes[grp]
                dtile = data_pool.tile([P, gs, C], F32, tag="data")
                eng = nc.sync if grp % 2 == 0 else nc.scalar
                eng.dma_start(out=dtile, in_=xv[:, starts[grp] : starts[grp] + gs, :])
                data_tiles[grp] = dtile

            def box_supergroup(sg):
                lo, hi = supergroups[sg]
                c0, c1 = starts[lo], starts[hi]
                ncols = c1 - c0
                # last supergroup on DVE: GpSimd must not be the post-stream
                # drain engine
                eng = nc.vector if sg == n_sg - 1 else nc.gpsimd
                e4 = scr_pool.tile([P, 2, ncols, 2], F32, tag="e4")
                u2 = scr_pool.tile([P, ncols, 2], F32, tag="u2")
                t01 = tgt[:, c0:c1, 0:2]
                t23 = tgt[:, c0:c1, 2:4]
                bp01 = box_all[:, c0:c1, 0:2]
                bp23 = box_all[:, c0:c1, 2:4]
                #   e_cx_cy = (t01 + t23) - 2*bp01      -> (0.5*e)^2 = err^2
                #   e_wh    = 2*((t23 - t01) - bp23)    -> (0.5*e)^2 = err^2
                eng.tensor_add(u2, t01, t23)
                eng.tensor_sub(u2, u2, bp01)
                eng.tensor_sub(e4[:, 0, :, :], u2, bp01)
                eng.tensor_sub(u2, t23, t01)
                eng.tensor_sub(u2, u2, bp23)
                eng.tensor_add(e4[:, 1, :, :], u2, u2)
                eng.tensor_mul(e4, e4, e4)
                # GpSimd can't do free-axis reduces; this one is tiny on DVE
                nc.vector.tensor_reduce(
                    out=loc_sg[:, sg : sg + 1], in_=e4,
                    axis=mybir.AxisListType.XYZ, op=ALU.add,
                )

            issue_dma(0)
            issue_dma(1)
            sg_idx = 0
            for grp, gs in enumerate(group_sizes):
                if grp + 2 < n_groups:
                    issue_dma(grp + 2)
                data = data_tiles[grp]
                t0 = starts[grp]

                # stage box-pred cols; batched math per supergroup
                nc.gpsimd.tensor_copy(box_all[:, t0 : t0 + gs, :], data[:, :, 0:4])

                for g in range(gs):
                    tt = t0 + g
                    exp_scr = scr_pool.tile([P, NCLS], F32, tag="exp_scr")
                    nc.scalar.activation(
                        out=exp_scr,
                        in_=data[:, g, 4:C],
                        func=ACTF.Exp,
                        accum_out=sumexp_all[:, tt : tt + 1],
                    )
                    pick_scr = scr_pool.tile([P, NCLS], F32, tag="pick_dve")
                    nc.vector.scalar_tensor_tensor(
                        pick_scr,
                        iota_t,
                        tgt[:, tt, 4:5],
                        data[:, g, 4:C],
                        ALU.is_equal,
                        ALU.mult,
                        accum_out=picked_all[:, tt : tt + 1],
                    )
                if grp + 1 == supergroups[sg_idx][1]:
                    box_supergroup(sg_idx)
                    sg_idx += 1
                if grp == epi_grp:
                    # mid-stream epilogue over cols [0, EPI_SPLIT): both
                    # engines are stream-paced here, this hides in their idle
                    nc.scalar.activation(
                        out=logz_scr[:, 0:EPI_SPLIT],
                        in_=sumexp_all[:, 0:EPI_SPLIT],
                        func=ACTF.Ln,
                        accum_out=logz_sumA,
                    )
                    nc.vector.tensor_reduce(
                        out=pick_sumA,
                        in_=picked_all[:, 0:EPI_SPLIT],
                        axis=mybir.AxisListType.X,
                        op=ALU.add,
                    )

            # ---- epilogue ----
            # s_pad is transposed in 32-blocks before the store so the DMA
            # writes 4 contiguous 128B runs (partitions 0/32/64/96) instead
            # of 128 scattered 4B writes (each a slow HBM read-modify-write;
            # the baseline paid ~7us of completion latency on this).
            s_pad = acc_pool.tile([P, 32], F32)
            nc.gpsimd.memset(s_pad, 0.0)
            nc.scalar.activation(
                out=logz_scr[:, EPI_SPLIT:T],
                in_=sumexp_all[:, EPI_SPLIT:T],
                func=ACTF.Ln,
                accum_out=logz_sumB,
            )
            nc.vector.tensor_reduce(
                out=pick_sumB,
                in_=picked_all[:, EPI_SPLIT:T],
                axis=mybir.AxisListType.X,
                op=ALU.add,
            )
            loc_sum = acc_pool.tile([P, 1], F32)
            nc.vector.tensor_reduce(
                out=loc_sum, in_=loc_sg, axis=mybir.AxisListType.X, op=ALU.add
            )
            s = acc_pool.tile([P, 1], F32)
            # loc_sg holds (2*err)^2 sums -> mean over 4 comps with the
            # doubling correction is 0.25 * 0.25 = 0.0625
            nc.vector.scalar_tensor_tensor(
                s, loc_sum, 0.0625, logz_sumA, ALU.mult, ALU.add
            )
            nc.vector.tensor_add(s, s, logz_sumB)
            nc.vector.tensor_sub(s, s, pick_sumA)
            nc.vector.tensor_sub(s_pad[:, 0:1], s, pick_sumB)
            s_t = acc_pool.tile([P, 32], F32)
            nc.vector.transpose(s_t, s_pad)
            nc.sync.dma_start(out=out[:], in_=s_t[0:P:32, :])

    return nc


def _run(output, target, **spmd_kwargs):
    output = np.ascontiguousarray(np.asarray(output, dtype=np.float32))
    target = np.ascontiguousarray(np.asarray(target, dtype=np.float32))
    assert output.shape == (B, C), output.shape
    assert target.shape == (B, 5), target.shape
    nc = build()
    in_maps = [
        {
            "x": output[i * R : (i + 1) * R],
            "t": target[i * R : (i + 1) * R],
        }
        for i in range(NCORES)
    ]
    res = run_bass_kernel_spmd(nc, in_maps, core_ids=list(range(NCORES)), **spmd_kwargs)
    total = 0.0
    for r in res.results:
        total += r["partial"].astype(np.float64).sum()
    return np.float32(total / B), res


def kernel(output, target):
    val, _ = _run(output, target)
    return np.asarray(val, dtype=np.float32)


def kernel_profiled(output, target, **kw):
    """Returns (scalar, BassKernelResults) with trace for perf analysis."""
    return _run(output, target, trace=True, **kw)
